# revision 22
# baseline (speedup 1.0000x reference)
"""HOG generator kernel for Trainium2, data-parallel over 8 NeuronCores.

v4: natural layout (no swizzle), 0/1 ratio-threshold masks.
Per tile = ONE image as [113p, (2, 226)f] (top|bottom halves side by
side, reflect-padded columns prepared on host in fp16).  Horizontal
sobel parts on DVE/Pool, vertical on PE fp16 banded matmuls.
rho = gx * recip_approx_fast(gy); 9 cumulative masks u_j = (rho >
tan_j - 1e-4) as contiguous DVE tensor_scalar is_gt ops (4x mode);
masked magnitudes via one big 2x tensor_tensor mul against the
broadcast gauss-weighted magnitude plane.  8:1 column pooling as a
3-stage pairwise add tree in natural layout (runs 4/2/1), 8:1 row
pooling on PE.  Bin histograms are adjacent differences of the
cumulative sums; wrap bin = T + W_8 - W_0.  L2-norm tail batched over
4 images.  fp16 device output; host converts + unfolds.
"""
import math
import sys

import numpy as np

sys.path.insert(0, "/opt/trn_rl_repo")

import concourse.bass as bass
import concourse.bacc as bacc
import concourse.mybir as mybir
from concourse import tile
from concourse import dve_ops as _dvo
from concourse.bass_utils import run_bass_kernel_spmd
from concourse.dve_spec import (
    AluOp as _AluOp, Spec as _Spec, Src0 as _Src0, Src1 as _Src1,
    C0 as _C0, Zero as _Zero, scan as _scan, select as _select,
    lower as _lower, _has_src1,
)
from concourse.dve_uop import DveOpSpec as _DveOpSpec


def _register_dve_op(name, spec):
    """Register a custom DVE op at runtime (sha computed on the fly)."""
    if name in _dvo._SUB_OPCODE_FOR_NAME:
        return next(o for o in _dvo.OPS if o.name == name)
    row = max(_dvo._SUB_OPCODE_FOR_NAME.values()) + 1
    assert row < 0x20, "custom DVE opcode rows exhausted"
    _dvo._SUB_OPCODE_FOR_NAME[name] = row
    shas = {}
    for ver in ("v3", "v4"):
        tmp = _DveOpSpec(name=name, opcode=row, uops=_lower(spec, ver=ver),
                         rd1_en=_has_src1(spec))
        shas[ver] = tmp.sha(ver)
    op = _dvo.DveOp(name, spec, subdim=False, uops_sha=shas)
    _dvo.OPS.append(op)
    _dvo.CUSTOM_DVE_SPECS[name] = spec
    return op


# out[k] = sum_{i<=k} (in0[i] > s0 ? in1[i] : 0) — masked-magnitude running
# sum; group sums fall out as strided diffs of the cumsum downstream.
_HOGPOOL = _register_dve_op("HOGPOOL", _Spec(
    body=_scan(_AluOp.ADD, _select(_Src0 > _C0, _Src1, _Zero)),
    reference=lambda in0, in1, s0, s1, imm2: np.cumsum(
        np.where(in0 > s0, in1, 0.0), axis=-1, dtype=np.float32)))
# out[k] = sum_{i<=k} in0[i]
_TCUMSUM = _register_dve_op("TCUMSUM", _Spec(
    body=_scan(_AluOp.ADD, _Src0),
    reference=lambda in0, in1, s0, s1, imm2: np.cumsum(
        in0, axis=-1, dtype=np.float32)))

N_CORES = 8
IMGS_PER_CORE = 16
GROUP = 4                 # images batched per normalization tail
NB = 9
F32 = mybir.dt.float32
F16 = mybir.dt.float16
AF = mybir.ActivationFunctionType
OP = mybir.AluOpType
TANS9 = [math.tan(j * math.pi / 9.0) for j in range(-4, 5)]


def _host_constants(weight_x, gaussian_kernel):
    g2 = np.asarray(gaussian_kernel, np.float64).reshape(16, 16)
    wt = np.sqrt(np.diag(g2)).astype(np.float64)   # g2[i,j] == wt[i]*wt[j]
    wx = np.asarray(weight_x, np.float32).reshape(3, 3)
    v_s = wx[:, 0].copy()                      # [1,2,1] vertical smooth
    v_d = wx[0, :].copy()                      # [1,0,-1] vertical diff

    def band(chunk, vec):
        m = np.zeros((113, 112), np.float32)
        for i in range(112):
            for dd in range(3):
                if chunk == 0:
                    r = i - 1 + dd
                    if r == -1:
                        r = 1
                else:
                    r = i + dd
                    if r == 113:
                        r = 111
                m[r, i] += vec[dd]
        return m

    poolm = np.zeros((112, 14), np.float32)
    for r in range(112):
        poolm[r, r // 8] = 1.0

    # full gaussian plane g[out_row, (h, c)] = wt_r * wt_c; both halves
    # share the row phase (output row i of either half is image row
    # i resp. 112+i, and 112 % 16 == 0)
    wr = wt[np.arange(112) % 16]
    wc = wt[np.arange(224) % 16]
    gpl = (wr[:, None] * np.tile(wc, 2)[None, :]).astype(np.float32)

    c16 = np.zeros((113, 910), np.float16)
    c16[:, 0:112] = band(0, v_s)
    c16[:, 112:224] = band(1, v_s)
    c16[:, 224:336] = band(0, v_d)
    c16[:, 336:448] = band(1, v_d)
    c16[0:112, 448:462] = poolm
    c16[0:112, 462:910] = gpl
    c32 = np.zeros((113, 1), np.float32)
    c32[0:14, 0] = 1e-8                       # eps bias for the norm sqrt
    return {"c16": c16, "c32": c32}


def _ap(t_ap, dims, off=0):
    """Build an AP on the same tensor with explicit [step, num] dims."""
    return bass.AP(t_ap.tensor, t_ap.offset + off, [list(d) for d in dims])


def build_program(n_img=IMGS_PER_CORE):
    nc = bacc.Bacc("TRN2", debug=False)
    x_d = nc.dram_tensor("x", [n_img, 113, 452], F16, kind="ExternalInput").ap()
    c16_d = nc.dram_tensor("c16", [113, 910], F16, kind="ExternalInput").ap()
    c32_d = nc.dram_tensor("c32", [113, 1], F32, kind="ExternalInput").ap()
    out_d = nc.dram_tensor("out", [n_img, 28, NB, 28], F16,
                           kind="ExternalOutput").ap()

    with tile.TileContext(nc) as tc:
        with (
            tc.tile_pool(name="const", bufs=1) as cp,
            tc.tile_pool(name="xin", bufs=3) as xp,
            tc.tile_pool(name="work", bufs=2) as wp,
            tc.tile_pool(name="big", bufs=2) as bp,
            tc.tile_pool(name="small", bufs=2) as sp,
            tc.tile_pool(name="bt", bufs=2) as btp,
            tc.tile_pool(name="tail", bufs=2) as tp,
            tc.tile_pool(name="psum", bufs=2, space="PSUM") as pp,
            tc.tile_pool(name="psum2", bufs=2, space="PSUM") as pp2,
        ):
            CT = cp.tile([113, 910], F16, tag="CT")
            nc.sync.dma_start(CT[:, :], c16_d)
            C32 = cp.tile([113, 1], F32, tag="C32")
            nc.sync.dma_start(C32[:, :], c32_d)
            # cumsum scratch: 10 planes of [pad0 | 448 running sums]
            # (fp32). Pad columns stay 0 across images (scans write cols
            # 1.. only).
            SGC = cp.tile([112, 10 * 449], F32, tag="SGC")
            nc.gpsimd.memset(
                _ap(SGC[:, :], [SGC[:, :].ap[0], [449, 10]]), 0.0)
            bs = [CT[:, 0:112], CT[:, 112:224]]
            bd = [CT[:, 224:336], CT[:, 336:448]]
            poolm_ap = CT[0:112, 448:462]
            gplane = CT[0:112, 462:910]        # [112, 448] gaussian plane
            eps_ap = C32[0:14, 0:1]

            def frontend(i0):
                """Load image i0 and compute horizontal sobel parts D, S."""
                X = xp.tile([113, 452], F16, tag="X")
                nc.scalar.dma_start(X[:, :], x_d[i0, :, :])
                xv = X[:, :]
                # D[c] = xp[c] - xp[c+2]  (Pool)
                D = wp.tile([113, 448], F16, tag="D")
                nc.gpsimd.tensor_sub(
                    _ap(D[:, :], [D[:, :].ap[0], [224, 2], [1, 224]]),
                    _ap(xv, [xv.ap[0], [226, 2], [1, 224]], off=0),
                    _ap(xv, [xv.ap[0], [226, 2], [1, 224]], off=2))
                # S[c] = 2*xp[c+1] + xp[c] + xp[c+2]
                # = (xp[c+1]+xp[c+2]) + (xp[c]+xp[c+1])  (Pool, 3 TTs)
                U = wp.tile([113, 448], F16, tag="U")
                uview = _ap(U[:, :], [U[:, :].ap[0], [224, 2], [1, 224]])
                nc.gpsimd.tensor_add(
                    uview,
                    _ap(xv, [xv.ap[0], [226, 2], [1, 224]], off=1),
                    _ap(xv, [xv.ap[0], [226, 2], [1, 224]], off=2))
                V = wp.tile([113, 448], F16, tag="V")
                vview = _ap(V[:, :], [V[:, :].ap[0], [224, 2], [1, 224]])
                nc.gpsimd.tensor_add(
                    vview,
                    _ap(xv, [xv.ap[0], [226, 2], [1, 224]], off=0),
                    _ap(xv, [xv.ap[0], [226, 2], [1, 224]], off=1))
                S = wp.tile([113, 448], F16, tag="S")
                nc.gpsimd.tensor_add(S[:, :], U[:, :], V[:, :])
                return D, S

            def emit_tail(BT, g):
                """L2-norm tail for one GROUP of images (deferred one group
                so its cross-engine chain overlaps the next group's work)."""
                # ---- batched tail over GROUP images ----
                bt = BT[:, :]
                b4 = [bt.ap[0], [560, GROUP], [1, 224]]
                b1 = [bt.ap[0], [560, GROUP], [1, 56]]
                HT = tp.tile([14, GROUP * 504], F16, tag="HT")
                ht = HT[:, :]
                h4 = [ht.ap[0], [504, GROUP], [1, 224]]
                h1 = [ht.ap[0], [504, GROUP], [1, 56]]
                # bins 5..8 = W(0..3) - W(1..4); bins 0..3 = W(4..7) - W(5..8)
                nc.gpsimd.tensor_sub(_ap(ht, h4, off=5 * 56),
                                     _ap(bt, b4, off=0),
                                     _ap(bt, b4, off=56))
                nc.gpsimd.tensor_sub(_ap(ht, h4, off=0),
                                     _ap(bt, b4, off=224),
                                     _ap(bt, b4, off=280))
                # bin 4 = T + W_8 - W_0
                TM = tp.tile([14, GROUP * 56], F16, tag="TM")
                tm = _ap(TM[:, :], [TM[:, :].ap[0], [56, GROUP], [1, 56]])
                nc.gpsimd.tensor_sub(tm, _ap(bt, b1, off=8 * 56),
                                     _ap(bt, b1, off=0))
                nc.gpsimd.tensor_add(_ap(ht, h1, off=4 * 56), tm,
                                     _ap(bt, b1, off=504))
                # ---- L2 normalize over the 9 bins ----
                SQ = tp.tile([14, GROUP * 504], F16, tag="SQ")
                sq = SQ[:, :]
                half = GROUP * 504 // 2
                nc.scalar.activation(SQ[:, 0:half], HT[:, 0:half], AF.Square)
                nc.scalar.activation(SQ[:, half:], HT[:, half:], AF.Square)
                s4 = [sq.ap[0], [504, GROUP], [1, 224]]
                s2d = [sq.ap[0], [504, GROUP], [1, 112]]
                s1 = [sq.ap[0], [504, GROUP], [1, 56]]
                SA = tp.tile([14, GROUP * 224], F16, tag="SA")
                sa = _ap(SA[:, :], [SA[:, :].ap[0], [224, GROUP], [1, 224]])
                nc.vector.tensor_add(sa, _ap(sq, s4, off=0),
                                     _ap(sq, s4, off=224))
                SB = tp.tile([14, GROUP * 112], F16, tag="SB")
                sb = _ap(SB[:, :], [SB[:, :].ap[0], [112, GROUP], [1, 112]])
                nc.vector.tensor_add(sb, _ap(sa, [sa.ap[0], [224, GROUP],
                                                  [1, 112]], off=0),
                                     _ap(sa, [sa.ap[0], [224, GROUP],
                                              [1, 112]], off=112))
                SC = tp.tile([14, GROUP * 56], F16, tag="SC")
                sc = _ap(SC[:, :], [SC[:, :].ap[0], [56, GROUP], [1, 56]])
                nc.vector.tensor_add(sc, _ap(sb, [sb.ap[0], [112, GROUP],
                                                  [1, 56]], off=0),
                                     _ap(sb, [sb.ap[0], [112, GROUP],
                                              [1, 56]], off=56))
                SS = tp.tile([14, GROUP * 56], F32, tag="SS")
                ssv = _ap(SS[:, :], [SS[:, :].ap[0], [56, GROUP], [1, 56]])
                nc.vector.tensor_add(ssv, sc, _ap(sq, s1, off=8 * 56))
                NRM = tp.tile([14, GROUP * 56], F32, tag="NRM")
                nc.scalar.activation(NRM[:, :], SS[:, :], AF.Sqrt,
                                     bias=eps_ap)
                INV = tp.tile([14, GROUP * 56], F32, tag="INV")
                with nc.allow_low_precision("approx reciprocal"):
                    nc.vector.reciprocal_approx_fast(INV[:, :], NRM[:, :])
                INV16 = tp.tile([14, GROUP * 56], F16, tag="INV16")
                nc.scalar.activation(INV16[:, :], INV[:, :], AF.Copy)
                OUTT = tp.tile([14, GROUP * 504], F16, tag="OUTT")
                ot = OUTT[:, :]
                gh = GROUP // 2
                for c in range(2):
                    nc.vector.tensor_mul(
                        _ap(ot, [ot.ap[0], [504, gh], [56, NB], [1, 56]],
                            off=c * gh * 504),
                        _ap(ht, [ht.ap[0], [504, gh], [56, NB], [1, 56]],
                            off=c * gh * 504),
                        _ap(INV16[:, :], [INV16[:, :].ap[0], [56, gh],
                                          [0, NB], [1, 56]],
                            off=c * gh * 56))

                # ---- store: per image [14,(k,h,c)] -> out[i, h*14+r, k, c]
                for m in range(GROUP):
                    i0 = g * GROUP + m
                    odst = bass.AP(out_d.tensor,
                                   out_d.offset + i0 * 28 * NB * 28,
                                   [[NB * 28, 14], [28, NB],
                                    [14 * NB * 28, 2], [1, 28]])
                    nc.sync.dma_start(
                        odst,
                        OUTT[:, m * 504:(m + 1) * 504].rearrange(
                            "p (k h c) -> p k h c", k=NB, h=2))

            fe = frontend(0)
            pending = None
            for g in range(n_img // GROUP):
                BT = btp.tile([14, GROUP * 560], F16, tag="BT")
                for m in range(GROUP):
                    i0 = g * GROUP + m
                    D, S = fe

                    # ---- vertical sobel on PE (fp16 banded matmuls) ----
                    gxp = pp.tile([112, 448], F32, tag="gx")
                    gyp = pp.tile([112, 448], F32, tag="gy")
                    for h in range(2):
                        nc.tensor.matmul(gxp[:, h * 224:(h + 1) * 224], bs[h],
                                         D[:, h * 224:(h + 1) * 224],
                                         start=True, stop=True)
                        nc.tensor.matmul(gyp[:, h * 224:(h + 1) * 224], bd[h],
                                         S[:, h * 224:(h + 1) * 224],
                                         start=True, stop=True)

                    # issue next image's load + horizontal sobel now so the
                    # Pool S-chain overlaps this image's DVE/ACT work
                    if i0 + 1 < n_img:
                        fe = frontend(i0 + 1)

                    # ---- magnitude * gauss; rho = gx / gy ----
                    # DVE op order follows dependency arrival: recip (needs
                    # gy only), S2 (after ACT squares), rho, t (after mag)
                    A = wp.tile([112, 448], F16, tag="A")
                    nc.scalar.activation(A[:, :], gxp[:, :], AF.Square)
                    Cq = wp.tile([112, 448], F16, tag="Cq")
                    nc.scalar.activation(Cq[:, :], gyp[:, :], AF.Square)
                    rgy = wp.tile([112, 448], F32, tag="rgy")
                    with nc.allow_low_precision("approx reciprocal"):
                        nc.vector.reciprocal_approx_fast(rgy[:, :], gyp[:, :])
                    S2 = wp.tile([112, 448], F16, tag="S2")
                    nc.vector.tensor_add(S2[:, :], A[:, :], Cq[:, :])
                    mag = wp.tile([112, 448], F16, tag="mag")
                    nc.scalar.activation(mag[:, :], S2[:, :], AF.Sqrt)
                    RHO = wp.tile([112, 448], F16, tag="RHO")
                    with nc.allow_low_precision("fp16 ratio"):
                        nc.vector.tensor_mul(RHO[:, :], gxp[:, :], rgy[:, :])
                    TPL = wp.tile([112, 448], F16, tag="TPL")
                    tpl = TPL[:, :]
                    nc.vector.tensor_mul(tpl, mag[:, :], gplane)

                    # ---- fused mask+mul+pool: per plane one custom DVE op
                    # ---- writes the running sum of (rho > T_j ? t : 0);
                    # ---- plane 9 = running sum of t itself ----
                    for j in range(NB):
                        nc.vector._custom_dve(
                            _HOGPOOL,
                            out=SGC[:, j * 449 + 1:(j + 1) * 449],
                            in0=RHO[:, :], in1=tpl, s0=TANS9[j] - 1e-4)
                    nc.vector._custom_dve(
                        _TCUMSUM, out=SGC[:, NB * 449 + 1:10 * 449],
                        in0=tpl)
                    # 8:1 column sums = strided diffs of the cumsums
                    CPt = sp.tile([112, 560], F16, tag="CP")
                    nc.vector.tensor_sub(
                        _ap(CPt[:, :], [CPt[:, :].ap[0], [56, 10], [1, 56]]),
                        _ap(SGC[:, :], [SGC[:, :].ap[0], [449, 10], [8, 56]],
                            off=8),
                        _ap(SGC[:, :], [SGC[:, :].ap[0], [449, 10], [8, 56]],
                            off=0))

                    # ---- 8:1 row pooling on PE ----
                    Pp1 = pp2.tile([14, NB * 56], F32, tag="Pp1")
                    nc.tensor.matmul(Pp1[:, :], poolm_ap, CPt[:, 0:NB * 56],
                                     start=True, stop=True)
                    Pp2 = pp2.tile([14, 56], F32, tag="Pp2")
                    nc.tensor.matmul(Pp2[:, :], poolm_ap, CPt[:, NB * 56:],
                                     start=True, stop=True)
                    nc.scalar.activation(BT[:, m * 560:m * 560 + 504],
                                         Pp1[:, :], AF.Copy)
                    nc.scalar.activation(BT[:, m * 560 + 504:(m + 1) * 560],
                                         Pp2[:, :], AF.Copy)

                emit_tail(BT, g)

    nc.compile()
    return nc


def _install_ntff_shim():
    """Provide antenv.axon_hooks (absent in this image) so trace=True works."""
    import sys as _sys
    if "antenv.axon_hooks" in _sys.modules:
        return
    import contextlib
    import ctypes
    import types

    so_path = "/opt/axon/libaxon_pjrt.so"
    lib = ctypes.CDLL(so_path)
    if not hasattr(lib, "axon_start_nrt_profile"):
        hook = None
    else:
        lib.axon_start_nrt_profile.argtypes = [
            ctypes.POINTER(ctypes.c_int64), ctypes.c_size_t]
        lib.axon_start_nrt_profile.restype = ctypes.c_int64
        lib.axon_stop_nrt_profile.argtypes = [ctypes.c_char_p]
        lib.axon_stop_nrt_profile.restype = ctypes.c_int64

        @contextlib.contextmanager
        def hook(output_dir, device_ids):
            import jax
            jax.devices()
            if device_ids:
                ids = (ctypes.c_int64 * len(device_ids))(*device_ids)
                rc = lib.axon_start_nrt_profile(ids, len(device_ids))
            else:
                rc = lib.axon_start_nrt_profile(None, 0)
            if rc != 0:
                raise RuntimeError(f"axon_start_nrt_profile rc={rc}")
            try:
                yield
            finally:
                n = lib.axon_stop_nrt_profile(str(output_dir).encode())
                print(f"profile: {n} file(s) written to {output_dir}",
                      file=sys.stderr)

    mod = types.ModuleType("antenv.axon_hooks")
    mod._hook = hook
    mod.get_axon_ntff_profile_hook = lambda: mod._hook
    mod.set_axon_ntff_profile_hook = lambda h: setattr(mod, "_hook", h)
    _sys.modules["antenv.axon_hooks"] = mod


_prog_cache = {}


def _get_prog(n_img):
    if n_img not in _prog_cache:
        _prog_cache[n_img] = build_program(n_img)
    return _prog_cache[n_img]


def kernel(x, weight_x, weight_y, gaussian_kernel, _trace=False):
    x = np.asarray(x, np.float32).reshape(128, 224, 224)
    # host-side fp16 convert + reflect-pad + two-halves layout
    x16 = x.astype(np.float16)
    xp = np.empty((128, 113, 452), np.float16)
    xp[:, :, 1:225] = x16[:, 0:113, :]
    xp[:, :, 227:451] = x16[:, 111:224, :]
    xp[:, :, 0] = x16[:, 0:113, 1]
    xp[:, :, 225] = x16[:, 0:113, 222]
    xp[:, :, 226] = x16[:, 111:224, 1]
    xp[:, :, 451] = x16[:, 111:224, 222]
    consts = _host_constants(weight_x, gaussian_kernel)
    nc = _get_prog(IMGS_PER_CORE)
    in_maps = []
    for c in range(N_CORES):
        m = {"x": xp[c * IMGS_PER_CORE:(c + 1) * IMGS_PER_CORE]}
        m.update(consts)
        in_maps.append(m)
    if _trace:
        _install_ntff_shim()
    res = run_bass_kernel_spmd(nc, in_maps, core_ids=list(range(N_CORES)),
                               trace=_trace)
    outs = [np.asarray(r["out"], np.float32) for r in res.results]
    full = np.concatenate(outs, axis=0)                # (128, 28, 9, 28)
    feat = full.transpose(0, 2, 1, 3)                  # (b, 9, 28, 28)
    feat = feat.transpose(0, 2, 3, 1)                  # (b, 28, 28, 9)
    feat = feat.reshape(128, 14, 2, 14, 2, NB)
    feat = feat.transpose(0, 1, 3, 5, 2, 4).reshape(128, 196, NB * 4)
    if _trace:
        return np.ascontiguousarray(feat), res
    return np.ascontiguousarray(feat)


# revision 23
# speedup vs baseline: 1.1985x; 1.1985x over previous
"""HOG generator kernel for Trainium2, data-parallel over 8 NeuronCores.

v4: natural layout (no swizzle), 0/1 ratio-threshold masks.
Per tile = ONE image as [113p, (2, 226)f] (top|bottom halves side by
side, reflect-padded columns prepared on host in fp16).  Horizontal
sobel parts on DVE/Pool, vertical on PE fp16 banded matmuls.
rho = gx * recip_approx_fast(gy); 9 cumulative masks u_j = (rho >
tan_j - 1e-4) as contiguous DVE tensor_scalar is_gt ops (4x mode);
masked magnitudes via one big 2x tensor_tensor mul against the
broadcast gauss-weighted magnitude plane.  8:1 column pooling as a
3-stage pairwise add tree in natural layout (runs 4/2/1), 8:1 row
pooling on PE.  Bin histograms are adjacent differences of the
cumulative sums; wrap bin = T + W_8 - W_0.  L2-norm tail batched over
4 images.  fp16 device output; host converts + unfolds.
"""
import math
import sys

import numpy as np

sys.path.insert(0, "/opt/trn_rl_repo")

import concourse.bass as bass
import concourse.bacc as bacc
import concourse.mybir as mybir
from concourse import tile
from concourse import dve_ops as _dvo
from concourse.bass_utils import run_bass_kernel_spmd
from concourse.dve_spec import (
    AluOp as _AluOp, Spec as _Spec, Src0 as _Src0, Src1 as _Src1,
    C0 as _C0, Zero as _Zero, scan as _scan, select as _select,
    lower as _lower, _has_src1,
)
from concourse.dve_uop import DveOpSpec as _DveOpSpec


def _register_dve_op(name, spec):
    """Register a custom DVE op at runtime (sha computed on the fly)."""
    if name in _dvo._SUB_OPCODE_FOR_NAME:
        return next(o for o in _dvo.OPS if o.name == name)
    row = max(_dvo._SUB_OPCODE_FOR_NAME.values()) + 1
    assert row < 0x20, "custom DVE opcode rows exhausted"
    _dvo._SUB_OPCODE_FOR_NAME[name] = row
    shas = {}
    for ver in ("v3", "v4"):
        tmp = _DveOpSpec(name=name, opcode=row, uops=_lower(spec, ver=ver),
                         rd1_en=_has_src1(spec))
        shas[ver] = tmp.sha(ver)
    op = _dvo.DveOp(name, spec, subdim=False, uops_sha=shas)
    _dvo.OPS.append(op)
    _dvo.CUSTOM_DVE_SPECS[name] = spec
    return op


# out[k] = sum_{i<=k} (in0[i] > s0 ? in1[i] : 0) — masked-magnitude running
# sum; group sums fall out as strided diffs of the cumsum downstream.
_HOGPOOL = _register_dve_op("HOGPOOL", _Spec(
    body=_scan(_AluOp.ADD, _select(_Src0 > _C0, _Src1, _Zero)),
    reference=lambda in0, in1, s0, s1, imm2: np.cumsum(
        np.where(in0 > s0, in1, 0.0), axis=-1, dtype=np.float32)))
# out[k] = sum_{i<=k} in0[i]
_TCUMSUM = _register_dve_op("TCUMSUM", _Spec(
    body=_scan(_AluOp.ADD, _Src0),
    reference=lambda in0, in1, s0, s1, imm2: np.cumsum(
        in0, axis=-1, dtype=np.float32)))

N_CORES = 8
IMGS_PER_CORE = 16
GROUP = 4                 # images batched per normalization tail
NB = 9
F32 = mybir.dt.float32
F16 = mybir.dt.float16
AF = mybir.ActivationFunctionType
OP = mybir.AluOpType
TANS9 = [math.tan(j * math.pi / 9.0) for j in range(-4, 5)]


def _host_constants(weight_x, gaussian_kernel):
    g2 = np.asarray(gaussian_kernel, np.float64).reshape(16, 16)
    wt = np.sqrt(np.diag(g2)).astype(np.float64)   # g2[i,j] == wt[i]*wt[j]
    wx = np.asarray(weight_x, np.float32).reshape(3, 3)
    v_s = wx[:, 0].copy()                      # [1,2,1] vertical smooth
    v_d = wx[0, :].copy()                      # [1,0,-1] vertical diff

    def band(chunk, vec):
        m = np.zeros((113, 112), np.float32)
        for i in range(112):
            for dd in range(3):
                if chunk == 0:
                    r = i - 1 + dd
                    if r == -1:
                        r = 1
                else:
                    r = i + dd
                    if r == 113:
                        r = 111
                m[r, i] += vec[dd]
        return m

    poolm = np.zeros((112, 14), np.float32)
    for r in range(112):
        poolm[r, r // 8] = 1.0

    # full gaussian plane g[out_row, (h, c)] = wt_r * wt_c; both halves
    # share the row phase (output row i of either half is image row
    # i resp. 112+i, and 112 % 16 == 0)
    wr = wt[np.arange(112) % 16]
    wc = wt[np.arange(224) % 16]
    gpl = (wr[:, None] * np.tile(wc, 2)[None, :]).astype(np.float32)

    c16 = np.zeros((113, 910), np.float16)
    c16[:, 0:112] = band(0, v_s)
    c16[:, 112:224] = band(1, v_s)
    c16[:, 224:336] = band(0, v_d)
    c16[:, 336:448] = band(1, v_d)
    c16[0:112, 448:462] = poolm
    c16[0:112, 462:910] = gpl
    c32 = np.zeros((113, 1), np.float32)
    c32[0:14, 0] = 1e-8                       # eps bias for the norm sqrt
    return {"c16": c16, "c32": c32}


def _ap(t_ap, dims, off=0):
    """Build an AP on the same tensor with explicit [step, num] dims."""
    return bass.AP(t_ap.tensor, t_ap.offset + off, [list(d) for d in dims])


def build_program(n_img=IMGS_PER_CORE):
    nc = bacc.Bacc("TRN2", debug=False)
    x_d = nc.dram_tensor("x", [n_img, 113, 452], F16, kind="ExternalInput").ap()
    c16_d = nc.dram_tensor("c16", [113, 910], F16, kind="ExternalInput").ap()
    c32_d = nc.dram_tensor("c32", [113, 1], F32, kind="ExternalInput").ap()
    out_d = nc.dram_tensor("out", [n_img, 28, NB, 28], F16,
                           kind="ExternalOutput").ap()

    with tile.TileContext(nc) as tc:
        with (
            tc.tile_pool(name="const", bufs=1) as cp,
            tc.tile_pool(name="xin", bufs=3) as xp,
            tc.tile_pool(name="work", bufs=2) as wp,
            tc.tile_pool(name="big", bufs=2) as bp,
            tc.tile_pool(name="small", bufs=2) as sp,
            tc.tile_pool(name="bt", bufs=2) as btp,
            tc.tile_pool(name="tail", bufs=2) as tp,
            tc.tile_pool(name="psum", bufs=2, space="PSUM") as pp,
            tc.tile_pool(name="psum2", bufs=2, space="PSUM") as pp2,
        ):
            CT = cp.tile([113, 910], F16, tag="CT")
            nc.sync.dma_start(CT[:, :], c16_d)
            C32 = cp.tile([113, 1], F32, tag="C32")
            nc.sync.dma_start(C32[:, :], c32_d)
            # cumsum scratch: 10 planes of [pad0 | 448 running sums]
            # (fp32). Pad columns stay 0 across images (scans write cols
            # 1.. only).
            SGC = cp.tile([112, 10 * 449], F32, tag="SGC")
            nc.gpsimd.memset(
                _ap(SGC[:, :], [SGC[:, :].ap[0], [449, 10]]), 0.0)
            bs = [CT[:, 0:112], CT[:, 112:224]]
            bd = [CT[:, 224:336], CT[:, 336:448]]
            poolm_ap = CT[0:112, 448:462]
            gplane = CT[0:112, 462:910]        # [112, 448] gaussian plane
            eps_ap = C32[0:14, 0:1]

            def frontend(i0):
                """Load image i0 and compute horizontal sobel parts D, S."""
                X = xp.tile([113, 452], F16, tag="X")
                nc.scalar.dma_start(X[:, :], x_d[i0, :, :])
                xv = X[:, :]
                # D[c] = xp[c] - xp[c+2]  (DVE, 2x)
                D = wp.tile([113, 448], F16, tag="D")
                nc.vector.tensor_sub(
                    _ap(D[:, :], [D[:, :].ap[0], [224, 2], [1, 224]]),
                    _ap(xv, [xv.ap[0], [226, 2], [1, 224]], off=0),
                    _ap(xv, [xv.ap[0], [226, 2], [1, 224]], off=2))
                # S[c] = 2*xp[c+1] + xp[c] + xp[c+2]
                # = (xp[c+1]+xp[c+2]) + (xp[c]+xp[c+1])  (Pool, 3 TTs)
                U = wp.tile([113, 448], F16, tag="U")
                uview = _ap(U[:, :], [U[:, :].ap[0], [224, 2], [1, 224]])
                nc.gpsimd.tensor_add(
                    uview,
                    _ap(xv, [xv.ap[0], [226, 2], [1, 224]], off=1),
                    _ap(xv, [xv.ap[0], [226, 2], [1, 224]], off=2))
                V = wp.tile([113, 448], F16, tag="V")
                vview = _ap(V[:, :], [V[:, :].ap[0], [224, 2], [1, 224]])
                nc.gpsimd.tensor_add(
                    vview,
                    _ap(xv, [xv.ap[0], [226, 2], [1, 224]], off=0),
                    _ap(xv, [xv.ap[0], [226, 2], [1, 224]], off=1))
                S = wp.tile([113, 448], F16, tag="S")
                nc.gpsimd.tensor_add(S[:, :], U[:, :], V[:, :])
                return D, S

            def emit_tail(BT, g):
                """L2-norm tail for one GROUP of images (deferred one group
                so its cross-engine chain overlaps the next group's work)."""
                # ---- batched tail over GROUP images ----
                bt = BT[:, :]
                b4 = [bt.ap[0], [560, GROUP], [1, 224]]
                b1 = [bt.ap[0], [560, GROUP], [1, 56]]
                HT = tp.tile([14, GROUP * 504], F16, tag="HT")
                ht = HT[:, :]
                h4 = [ht.ap[0], [504, GROUP], [1, 224]]
                h1 = [ht.ap[0], [504, GROUP], [1, 56]]
                # bins 5..8 = W(0..3) - W(1..4); bins 0..3 = W(4..7) - W(5..8)
                nc.gpsimd.tensor_sub(_ap(ht, h4, off=5 * 56),
                                     _ap(bt, b4, off=0),
                                     _ap(bt, b4, off=56))
                nc.gpsimd.tensor_sub(_ap(ht, h4, off=0),
                                     _ap(bt, b4, off=224),
                                     _ap(bt, b4, off=280))
                # bin 4 = T + W_8 - W_0
                TM = tp.tile([14, GROUP * 56], F16, tag="TM")
                tm = _ap(TM[:, :], [TM[:, :].ap[0], [56, GROUP], [1, 56]])
                nc.gpsimd.tensor_sub(tm, _ap(bt, b1, off=8 * 56),
                                     _ap(bt, b1, off=0))
                nc.gpsimd.tensor_add(_ap(ht, h1, off=4 * 56), tm,
                                     _ap(bt, b1, off=504))
                # ---- L2 normalize over the 9 bins ----
                SQ = tp.tile([14, GROUP * 504], F16, tag="SQ")
                sq = SQ[:, :]
                half = GROUP * 504 // 2
                nc.scalar.activation(SQ[:, 0:half], HT[:, 0:half], AF.Square)
                nc.scalar.activation(SQ[:, half:], HT[:, half:], AF.Square)
                s4 = [sq.ap[0], [504, GROUP], [1, 224]]
                s2d = [sq.ap[0], [504, GROUP], [1, 112]]
                s1 = [sq.ap[0], [504, GROUP], [1, 56]]
                SA = tp.tile([14, GROUP * 224], F16, tag="SA")
                sa = _ap(SA[:, :], [SA[:, :].ap[0], [224, GROUP], [1, 224]])
                nc.vector.tensor_add(sa, _ap(sq, s4, off=0),
                                     _ap(sq, s4, off=224))
                SB = tp.tile([14, GROUP * 112], F16, tag="SB")
                sb = _ap(SB[:, :], [SB[:, :].ap[0], [112, GROUP], [1, 112]])
                nc.vector.tensor_add(sb, _ap(sa, [sa.ap[0], [224, GROUP],
                                                  [1, 112]], off=0),
                                     _ap(sa, [sa.ap[0], [224, GROUP],
                                              [1, 112]], off=112))
                SC = tp.tile([14, GROUP * 56], F16, tag="SC")
                sc = _ap(SC[:, :], [SC[:, :].ap[0], [56, GROUP], [1, 56]])
                nc.vector.tensor_add(sc, _ap(sb, [sb.ap[0], [112, GROUP],
                                                  [1, 56]], off=0),
                                     _ap(sb, [sb.ap[0], [112, GROUP],
                                              [1, 56]], off=56))
                SS = tp.tile([14, GROUP * 56], F32, tag="SS")
                ssv = _ap(SS[:, :], [SS[:, :].ap[0], [56, GROUP], [1, 56]])
                nc.vector.tensor_add(ssv, sc, _ap(sq, s1, off=8 * 56))
                NRM = tp.tile([14, GROUP * 56], F32, tag="NRM")
                nc.scalar.activation(NRM[:, :], SS[:, :], AF.Sqrt,
                                     bias=eps_ap)
                INV = tp.tile([14, GROUP * 56], F32, tag="INV")
                with nc.allow_low_precision("approx reciprocal"):
                    nc.vector.reciprocal_approx_fast(INV[:, :], NRM[:, :])
                INV16 = tp.tile([14, GROUP * 56], F16, tag="INV16")
                nc.scalar.activation(INV16[:, :], INV[:, :], AF.Copy)
                OUTT = tp.tile([14, GROUP * 504], F16, tag="OUTT")
                ot = OUTT[:, :]
                gh = GROUP // 2
                for c in range(2):
                    nc.vector.tensor_mul(
                        _ap(ot, [ot.ap[0], [504, gh], [56, NB], [1, 56]],
                            off=c * gh * 504),
                        _ap(ht, [ht.ap[0], [504, gh], [56, NB], [1, 56]],
                            off=c * gh * 504),
                        _ap(INV16[:, :], [INV16[:, :].ap[0], [56, gh],
                                          [0, NB], [1, 56]],
                            off=c * gh * 56))

                # ---- store: per image [14,(k,h,c)] -> out[i, h*14+r, k, c]
                for m in range(GROUP):
                    i0 = g * GROUP + m
                    odst = bass.AP(out_d.tensor,
                                   out_d.offset + i0 * 28 * NB * 28,
                                   [[NB * 28, 14], [28, NB],
                                    [14 * NB * 28, 2], [1, 28]])
                    nc.sync.dma_start(
                        odst,
                        OUTT[:, m * 504:(m + 1) * 504].rearrange(
                            "p (k h c) -> p k h c", k=NB, h=2))

            fe = frontend(0)
            pending = None
            for g in range(n_img // GROUP):
                BT = btp.tile([14, GROUP * 560], F16, tag="BT")
                for m in range(GROUP):
                    i0 = g * GROUP + m
                    D, S = fe

                    # ---- vertical sobel on PE (fp16 banded matmuls) ----
                    gxp = pp.tile([112, 448], F32, tag="gx")
                    gyp = pp.tile([112, 448], F32, tag="gy")
                    for h in range(2):
                        nc.tensor.matmul(gxp[:, h * 224:(h + 1) * 224], bs[h],
                                         D[:, h * 224:(h + 1) * 224],
                                         start=True, stop=True)
                        nc.tensor.matmul(gyp[:, h * 224:(h + 1) * 224], bd[h],
                                         S[:, h * 224:(h + 1) * 224],
                                         start=True, stop=True)

                    # issue next image's load + horizontal sobel now so the
                    # Pool S-chain overlaps this image's DVE/ACT work
                    if i0 + 1 < n_img:
                        fe = frontend(i0 + 1)

                    # ---- magnitude * gauss; rho = gx / gy ----
                    # DVE op order follows dependency arrival: recip (needs
                    # gy only), S2 (after ACT squares), rho, t (after mag)
                    A = wp.tile([112, 448], F16, tag="A")
                    nc.scalar.activation(A[:, :], gxp[:, :], AF.Square)
                    Cq = wp.tile([112, 448], F16, tag="Cq")
                    nc.scalar.activation(Cq[:, :], gyp[:, :], AF.Square)
                    rgy = wp.tile([112, 448], F32, tag="rgy")
                    with nc.allow_low_precision("approx reciprocal"):
                        nc.vector.reciprocal_approx_fast(rgy[:, :], gyp[:, :])
                    S2 = wp.tile([112, 448], F16, tag="S2")
                    nc.vector.tensor_add(S2[:, :], A[:, :], Cq[:, :])
                    mag = wp.tile([112, 448], F16, tag="mag")
                    nc.scalar.activation(mag[:, :], S2[:, :], AF.Sqrt)
                    RHO = wp.tile([112, 448], F16, tag="RHO")
                    with nc.allow_low_precision("fp16 ratio"):
                        nc.vector.tensor_mul(RHO[:, :], gxp[:, :], rgy[:, :])
                    TPL = wp.tile([112, 448], F16, tag="TPL")
                    tpl = TPL[:, :]
                    nc.vector.tensor_mul(tpl, mag[:, :], gplane)

                    # ---- fused mask+mul+pool: per plane one custom DVE op
                    # ---- writes the running sum of (rho > T_j ? t : 0);
                    # ---- plane 9 = running sum of t itself ----
                    for j in range(NB):
                        nc.vector._custom_dve(
                            _HOGPOOL,
                            out=SGC[:, j * 449 + 1:(j + 1) * 449],
                            in0=RHO[:, :], in1=tpl, s0=TANS9[j] - 1e-4)
                    nc.vector._custom_dve(
                        _TCUMSUM, out=SGC[:, NB * 449 + 1:10 * 449],
                        in0=tpl)
                    # 8:1 column sums = strided diffs of the cumsums
                    CPt = sp.tile([112, 560], F16, tag="CP")
                    nc.vector.tensor_sub(
                        _ap(CPt[:, :], [CPt[:, :].ap[0], [56, 10], [1, 56]]),
                        _ap(SGC[:, :], [SGC[:, :].ap[0], [449, 10], [8, 56]],
                            off=8),
                        _ap(SGC[:, :], [SGC[:, :].ap[0], [449, 10], [8, 56]],
                            off=0))

                    # ---- 8:1 row pooling on PE ----
                    Pp1 = pp2.tile([14, NB * 56], F32, tag="Pp1")
                    nc.tensor.matmul(Pp1[:, :], poolm_ap, CPt[:, 0:NB * 56],
                                     start=True, stop=True)
                    Pp2 = pp2.tile([14, 56], F32, tag="Pp2")
                    nc.tensor.matmul(Pp2[:, :], poolm_ap, CPt[:, NB * 56:],
                                     start=True, stop=True)
                    nc.scalar.activation(BT[:, m * 560:m * 560 + 504],
                                         Pp1[:, :], AF.Copy)
                    nc.scalar.activation(BT[:, m * 560 + 504:(m + 1) * 560],
                                         Pp2[:, :], AF.Copy)

                emit_tail(BT, g)

    nc.compile()
    return nc


def _install_ntff_shim():
    """Provide antenv.axon_hooks (absent in this image) so trace=True works."""
    import sys as _sys
    if "antenv.axon_hooks" in _sys.modules:
        return
    import contextlib
    import ctypes
    import types

    so_path = "/opt/axon/libaxon_pjrt.so"
    lib = ctypes.CDLL(so_path)
    if not hasattr(lib, "axon_start_nrt_profile"):
        hook = None
    else:
        lib.axon_start_nrt_profile.argtypes = [
            ctypes.POINTER(ctypes.c_int64), ctypes.c_size_t]
        lib.axon_start_nrt_profile.restype = ctypes.c_int64
        lib.axon_stop_nrt_profile.argtypes = [ctypes.c_char_p]
        lib.axon_stop_nrt_profile.restype = ctypes.c_int64

        @contextlib.contextmanager
        def hook(output_dir, device_ids):
            import jax
            jax.devices()
            if device_ids:
                ids = (ctypes.c_int64 * len(device_ids))(*device_ids)
                rc = lib.axon_start_nrt_profile(ids, len(device_ids))
            else:
                rc = lib.axon_start_nrt_profile(None, 0)
            if rc != 0:
                raise RuntimeError(f"axon_start_nrt_profile rc={rc}")
            try:
                yield
            finally:
                n = lib.axon_stop_nrt_profile(str(output_dir).encode())
                print(f"profile: {n} file(s) written to {output_dir}",
                      file=sys.stderr)

    mod = types.ModuleType("antenv.axon_hooks")
    mod._hook = hook
    mod.get_axon_ntff_profile_hook = lambda: mod._hook
    mod.set_axon_ntff_profile_hook = lambda h: setattr(mod, "_hook", h)
    _sys.modules["antenv.axon_hooks"] = mod


_prog_cache = {}


def _get_prog(n_img):
    if n_img not in _prog_cache:
        _prog_cache[n_img] = build_program(n_img)
    return _prog_cache[n_img]


def kernel(x, weight_x, weight_y, gaussian_kernel, _trace=False):
    x = np.asarray(x, np.float32).reshape(128, 224, 224)
    # host-side fp16 convert + reflect-pad + two-halves layout
    x16 = x.astype(np.float16)
    xp = np.empty((128, 113, 452), np.float16)
    xp[:, :, 1:225] = x16[:, 0:113, :]
    xp[:, :, 227:451] = x16[:, 111:224, :]
    xp[:, :, 0] = x16[:, 0:113, 1]
    xp[:, :, 225] = x16[:, 0:113, 222]
    xp[:, :, 226] = x16[:, 111:224, 1]
    xp[:, :, 451] = x16[:, 111:224, 222]
    consts = _host_constants(weight_x, gaussian_kernel)
    nc = _get_prog(IMGS_PER_CORE)
    in_maps = []
    for c in range(N_CORES):
        m = {"x": xp[c * IMGS_PER_CORE:(c + 1) * IMGS_PER_CORE]}
        m.update(consts)
        in_maps.append(m)
    if _trace:
        _install_ntff_shim()
    res = run_bass_kernel_spmd(nc, in_maps, core_ids=list(range(N_CORES)),
                               trace=_trace)
    outs = [np.asarray(r["out"], np.float32) for r in res.results]
    full = np.concatenate(outs, axis=0)                # (128, 28, 9, 28)
    feat = full.transpose(0, 2, 1, 3)                  # (b, 9, 28, 28)
    feat = feat.transpose(0, 2, 3, 1)                  # (b, 28, 28, 9)
    feat = feat.reshape(128, 14, 2, 14, 2, NB)
    feat = feat.transpose(0, 1, 3, 5, 2, 4).reshape(128, 196, NB * 4)
    if _trace:
        return np.ascontiguousarray(feat), res
    return np.ascontiguousarray(feat)
